# revision 23
# baseline (speedup 1.0000x reference)
"""Trainium2 Bass kernel for nn_AttentionMap (B=4, S=4096, D=256 full attention).

Sharding: 8 cores = 4 batches x 2 query-halves (data-parallel batch,
sequence-parallel over query rows, softmax rows stay whole per core).
No collectives: core c computes out[c//2, (c%2)*2048:(c%2+1)*2048, :]
from conv_local[c//2] and its conv_global slice.

Host-side preprocessing inside kernel() (same uploaded bytes, big device
savings):
  - X and G are uploaded TRANSPOSED (X^T [d, s], G^T [d, q]) so the device
    never runs the 96 PE transposes + 96 PSUM-drain copies the naive layout
    needs (they dominated the prologue).
  - The score weights are folded on the host in f32: M~ = Wq Wk^T [i, a]
    and b~ = Wk bq, so K and Q are never materialized; bk drops out
    entirely (softmax cancels a per-row constant).
  - All big tensors are cast to bf16 on the host (halves the axon-tunnel
    upload; the device would cast to bf16 for the PE anyway).

Device math (per core; every matmul contracts over the partition dim):
  Y^T[a, q] = M~T.T @ G^T + b~        (a-chunks of 128, q in tiles of 512)
  V[s, :256] = X^T-chunks.T @ Wv + bv ; V[s, 256:258] = 1  (ones-columns)
  per q-tile of 512 rows, software-pipelined with the previous tile's PV:
    S^T[s, q] = X^T-chunk.T @ Y^T-tile   (PSUM f32, 2 kv-chunks per 2-bank tile)
    p = exp(S^T / 16)  (ACT, bf16 out; no max-subtraction: scores ~ N(0,1)
        so bf16/f32 range is safe, softmax shift-invariance keeps results
        aligned with the reference)
    O_unnorm[q, 258] += p-chunk.T @ V-chunk  (4 PSUM accumulators; the
        ones-columns of V accumulate the softmax denominator)
    out = O_unnorm[:, :256] * reciprocal(O_unnorm[:, 256])  -> bf16 -> DMA
  The PV pass of q-tile i is interleaved between the S^T pairs of q-tile
  i+1 so the PE stays dense while ACT works through the exp backlog.

All matmuls are bf16 (fp8e4 DoubleRow was tried and is 1.9x faster on the
PE, but its ~3.6% operand quantization puts ~eps*w_max*|v| ~ 2-4e-2 absmax
error on rows where softmax concentrates - over the 2e-2 gate).

Output is bf16 (halves download), cast back to f32 on the host. The jitted
shard_map executable is built once and cached; donated output buffers are
created on-device instead of uploading host zeros.
"""

import os
import sys
from contextlib import ExitStack

import ml_dtypes
import numpy as np

for _p in ("/opt/trn_rl_repo", "/root/.axon_site/_ro/trn_rl_repo"):
    if _p not in sys.path and os.path.isdir(_p):
        sys.path.append(_p)

import concourse.mybir as mybir
import concourse.tile as tile
from concourse import bacc

B = 4
S = 4096          # kv sequence length (= full query length)
D = 256           # model dim = head dim
NCORES = 8
SQH = S // 2      # query rows per core (2048)
QT = 512          # query tile (moving free dim of the S^T matmuls)
NQT = SQH // QT   # 4
NSC = S // 128    # 32 kv chunks of 128
NDC = D // 128    # 2 d chunks of 128
VPAD = 2          # ones-columns appended to V (softmax denominator)
NXSEG = 2         # xt/gt DMA split for load/compute overlap (4KB packets)
F32 = mybir.dt.float32
BF16 = mybir.dt.bfloat16
I32 = mybir.dt.int32

# Schraudolph fast-exp on DVE for a subset of kv-chunks (ACT exp is the
# phase-2 wall otherwise): p ~= bitcast_f32(int32(s*EXPA + EXPB)) with
# max rel err ~3.1%; the softmax ratio cancels the common factor so the
# end-to-end absmax cost is a few 1e-3. Chunks with t % 4 == 2 go to DVE.
EXP_SCALE = 1.0 / 16.0          # 1/sqrt(D)
EXPA = float(2 ** 23) * float(np.log2(np.e)) * EXP_SCALE
EXPB = float(2 ** 23) * (127.0 - 0.0450)


def _dve_exp_chunk(t):
    return t % 4 == 2

_CACHED = {}


def build_program():
    nc = bacc.Bacc("TRN2", target_bir_lowering=False, debug=False)

    xt_d = nc.dram_tensor("xt", [D, S], BF16, kind="ExternalInput").ap()
    gt_d = nc.dram_tensor("gt", [D, SQH], BF16, kind="ExternalInput").ap()
    mt_d = nc.dram_tensor("mt", [D, D], BF16, kind="ExternalInput").ap()
    wv_d = nc.dram_tensor("wv", [D, D], BF16, kind="ExternalInput").ap()
    bt_d = nc.dram_tensor("bt", [D, 1], F32, kind="ExternalInput").ap()
    bv_d = nc.dram_tensor("bv", [1, D], F32, kind="ExternalInput").ap()
    out_d = nc.dram_tensor("out", [SQH, D], BF16, kind="ExternalOutput").ap()

    with tile.TileContext(nc) as tc, ExitStack() as ctx:
        Exp = mybir.ActivationFunctionType.Exp

        consts = ctx.enter_context(tc.tile_pool(name="consts", bufs=1))
        big = ctx.enter_context(tc.tile_pool(name="big", bufs=1))

        # ---- phase 2 SBUF residents ----
        xt = big.tile([128, NDC, S], BF16)          # X^T [d, s]
        gt = big.tile([128, NDC, SQH], BF16)        # G^T [d, q]
        yt = big.tile([128, NDC, SQH], BF16)        # M~T.T G^T + b~  [a, q]
        vt = big.tile([128, NSC, D + VPAD], BF16)   # V||1 [s, d+pad]

        mt_sb = consts.tile([128, NDC, D], BF16)    # M~T [i, a]
        wv_sb = consts.tile([128, NDC, D], BF16)
        bt_sb = consts.tile([128, NDC, 1], F32)     # b~ [a]
        ones1 = consts.tile([1, 128], BF16)
        ones1_f32 = consts.tile([1, 128], F32)
        vone_f32 = consts.tile([128, NSC, VPAD], F32)
        bv_bc = consts.tile([128, D], F32)

        # DMA across 3 queues (sync/scalar/gpsimd — the only DMA-capable
        # engines): gt first (it gates Y^T and thus the first score matmul),
        # weights on gpsimd, then xt in progressive 512-col segments so the
        # V projection can chase the stream
        bv_ld = consts.tile([1, D], F32, tag="bvl")
        nc.gpsimd.dma_start(bv_ld[:], bv_d[:])
        for kc in range(NDC):
            nc.gpsimd.dma_start(mt_sb[:, kc, :], mt_d[kc * 128:(kc + 1) * 128, :])
            nc.gpsimd.dma_start(wv_sb[:, kc, :], wv_d[kc * 128:(kc + 1) * 128, :])
            nc.gpsimd.dma_start(bt_sb[:, kc, :], bt_d[kc * 128:(kc + 1) * 128, :])
        xq = (nc.sync, nc.scalar)
        for kc in range(NDC):
            xq[kc].dma_start(xt[:, kc, 0:512], xt_d[kc * 128:(kc + 1) * 128, 0:512])
        for sg in range(4):
            for kc in range(NDC):
                xq[kc].dma_start(
                    gt[:, kc, sg * 512:(sg + 1) * 512],
                    gt_d[kc * 128:(kc + 1) * 128, sg * 512:(sg + 1) * 512])
        for sg in range(1, 8):
            for kc in range(NDC):
                xq[kc].dma_start(
                    xt[:, kc, sg * 512:(sg + 1) * 512],
                    xt_d[kc * 128:(kc + 1) * 128, sg * 512:(sg + 1) * 512])
        bv_rhs = consts.tile([1, D], BF16, tag="bvc")
        nc.vector.tensor_copy(bv_rhs[:], bv_ld[:])
        nc.vector.memset(ones1_f32[:], 1.0)
        nc.vector.tensor_copy(ones1[:], ones1_f32[:])
        nc.vector.memset(vone_f32[:], 1.0)

        p01 = ExitStack()
        with p01:
            mmp = p01.enter_context(tc.tile_pool(name="mmp", bufs=2, space="PSUM"))

            # bv broadcast across partitions via a K=1 matmul
            psb = mmp.tile([128, D], F32, tag="proj")
            nc.tensor.matmul(psb[:], ones1[:], bv_rhs[:], start=True, stop=True)
            nc.vector.tensor_copy(bv_bc[:], psb[:])
            nc.vector.tensor_copy(vt[:, :, D:D + VPAD], vone_f32[:])

            # Y^T block 0 only — it gates the first score matmul; blocks 1-3
            # are emitted inside the slot pipeline
            for dc in range(NDC):
                psy = mmp.tile([128, 512], F32, tag="proj", name="psy")
                for ic in range(NDC):
                    nc.tensor.matmul(
                        psy[:],
                        mt_sb[:, ic, dc * 128:(dc + 1) * 128],
                        gt[:, ic, 0:512],
                        start=(ic == 0), stop=(ic == NDC - 1),
                    )
                nc.vector.tensor_scalar_add(
                    yt[:, dc, 0:512], psy[:], bt_sb[:, dc, :])

        # ---- phase 2: attention, PV software-pipelined one q-tile behind ----
        esp = ctx.enter_context(tc.tile_pool(name="esp", bufs=2))
        stp = ctx.enter_context(tc.tile_pool(name="stp", bufs=3, space="PSUM"))
        pvp = ctx.enter_context(tc.tile_pool(name="pvp", bufs=1, space="PSUM"))
        osb_p = ctx.enter_context(tc.tile_pool(name="osb", bufs=4))

        inv_sqrt_d = 1.0 / float(np.sqrt(D))
        nqs = QT // 128

        yip = ctx.enter_context(tc.tile_pool(name="yip", bufs=3))

        def emit_scores_pair(es, q0, tp):
            # single-bank psum tiles (bufs=4): 4-deep PE->ACT pipelining; a
            # 2-bank/bufs=2 pair tile ping-pongs PE and ACT (~12 us/qtile)
            for sub in range(2):
                t = 2 * tp + sub
                ps = stp.tile([128, QT], F32, tag="st")
                for kc in range(NDC):
                    nc.tensor.matmul(
                        ps[:],
                        xt[:, kc, t * 128:(t + 1) * 128],
                        yt[:, kc, q0:q0 + QT],
                        start=(kc == 0), stop=(kc == NDC - 1),
                    )
                if _dve_exp_chunk(t):
                    yi = yip.tile([128, QT], I32, tag="yi")
                    nc.vector.tensor_scalar(
                        yi[:], ps[:], EXPA, EXPB,
                        op0=mybir.AluOpType.mult, op1=mybir.AluOpType.add)
                    nc.vector.tensor_copy(es[:, t, :], yi[:].bitcast(F32))
                else:
                    nc.scalar.activation(es[:, t, :], ps[:], Exp,
                                         scale=inv_sqrt_d)

        def emit_pv_pair(accs, es, tp):
            for qs in range(nqs):
                for t in (2 * tp, 2 * tp + 1):
                    nc.tensor.matmul(
                        accs[qs][:],
                        es[:, t, qs * 128:(qs + 1) * 128],
                        vt[:, t, :],
                        start=(t == 0), stop=(t == NSC - 1),
                    )

        def emit_finalize(accs, q0):
            for qs in range(nqs):
                acc = accs[qs]
                osb = osb_p.tile([128, D], BF16, tag="osb")
                rec = osb_p.tile([128, 1], F32, tag="rec")
                nc.vector.reciprocal(rec[:], acc[:, D:D + 1])
                nc.vector.tensor_scalar_mul(osb[:], acc[:, 0:D], rec[:])
                nc.gpsimd.dma_start(
                    out_d[q0 + qs * 128:q0 + (qs + 1) * 128, :], osb[:])

        def emit_vproj_chunk(t):
            # V[t] = X_t @ Wv + bv — interleaved into q-tile 0's slots; the
            # shared single-bank "pp" psum drains on DVE a slot before reuse
            psv = pvp.tile([128, D], F32, tag="pp", name="psv")
            for kc in range(NDC):
                nc.tensor.matmul(
                    psv[:],
                    xt[:, kc, t * 128:(t + 1) * 128],
                    wv_sb[:, kc, :],
                    start=(kc == 0), stop=(kc == NDC - 1),
                )
            nc.vector.tensor_add(vt[:, t, 0:D], psv[:], bv_bc[:])

        def emit_yt_half(nt, dc):
            psy = pvp.tile([128, 512], F32, tag="pp", name="psy")
            for ic in range(NDC):
                nc.tensor.matmul(
                    psy[:],
                    mt_sb[:, ic, dc * 128:(dc + 1) * 128],
                    gt[:, ic, nt * 512:(nt + 1) * 512],
                    start=(ic == 0), stop=(ic == NDC - 1),
                )
            nc.vector.tensor_scalar_add(
                yt[:, dc, nt * 512:(nt + 1) * 512], psy[:], bt_sb[:, dc, :])

        # flat slot pipeline: PV trails scores by PVLAG slots (same q-tile),
        # so the end-of-kernel drain is PVLAG slots instead of a whole tile.
        # q-tile 0's slots also carry the V projection (2 chunks/slot, 4
        # slots ahead of the PV consumer) and yt blocks 1-3 land just in
        # time for their q-tile.
        PVLAG = 4
        NSLOT = NSC // 2
        state = {}  # qi -> (es, accs, q0)
        for s in range(NQT * NSLOT + PVLAG):
            if s < NQT * NSLOT:
                qi, tp = divmod(s, NSLOT)
                if tp == 0:
                    es = esp.tile([128, NSC, QT], BF16, tag="es", name="es")
                    accs = [pvp.tile([128, D + VPAD], F32, tag=f"acc{qs}",
                                     name=f"acc{qs}") for qs in range(nqs)]
                    state[qi] = (es, accs, qi * QT)
                emit_scores_pair(state[qi][0], state[qi][2], tp)
                if qi == 0:
                    emit_vproj_chunk(2 * tp)
            if s >= PVLAG:
                qj, tq = divmod(s - PVLAG, NSLOT)
                emit_pv_pair(state[qj][1], state[qj][0], tq)
                if tq == NSLOT - 1:
                    emit_finalize(state[qj][1], state[qj][2])
                    del state[qj]
            if s < NQT * NSLOT:
                qi, tp = divmod(s, NSLOT)
                if qi == 0:
                    emit_vproj_chunk(2 * tp + 1)
                    if tp in (11, 13, 15):
                        nt = (tp - 11) // 2 + 1
                        emit_yt_half(nt, 0)
                        emit_yt_half(nt, 1)
                elif qi == 1 and tp in (2, 3):
                    pass

    nc.compile()
    return nc


def _get_exec():
    if "fn" in _CACHED:
        return _CACHED

    import jax
    import jax.numpy as jnp
    from jax.sharding import Mesh, PartitionSpec
    from jax.experimental.shard_map import shard_map
    from concourse import bass2jax

    nc = build_program()
    bass2jax.install_neuronx_cc_hook()

    partition_name = nc.partition_id_tensor.name if nc.partition_id_tensor else None
    in_names, out_names, out_avals = [], [], []
    for alloc in nc.m.functions[0].allocations:
        if not isinstance(alloc, mybir.MemoryLocationSet):
            continue
        name = alloc.memorylocations[0].name
        if alloc.kind == "ExternalInput":
            if name != partition_name:
                in_names.append(name)
        elif alloc.kind == "ExternalOutput":
            out_names.append(name)
            out_avals.append(jax.core.ShapedArray(
                tuple(alloc.tensor_shape), mybir.dt.np(alloc.dtype)))
    n_params = len(in_names)
    n_outs = len(out_avals)
    in_names_all = in_names + out_names
    if partition_name is not None:
        in_names_all.append(partition_name)
    donate = tuple(range(n_params, n_params + n_outs))

    def _body(*args):
        operands = list(args)
        if partition_name is not None:
            operands.append(bass2jax.partition_id_tensor())
        return tuple(bass2jax._bass_exec_p.bind(
            *operands,
            out_avals=tuple(out_avals),
            in_names=tuple(in_names_all),
            out_names=tuple(out_names),
            lowering_input_output_aliases=(),
            sim_require_finite=True,
            sim_require_nnan=True,
            nc=nc,
        ))

    devices = jax.devices()[:NCORES]
    mesh = Mesh(np.asarray(devices), ("core",))
    in_specs = (PartitionSpec("core"),) * (n_params + n_outs)
    out_specs = (PartitionSpec("core"),) * n_outs
    fn = jax.jit(
        shard_map(_body, mesh=mesh, in_specs=in_specs, out_specs=out_specs,
                  check_rep=False),
        donate_argnums=donate, keep_unused=True,
    )

    zero_info = [(tuple(a.shape), a.dtype) for a in out_avals]

    def _mkz():
        return tuple(jnp.zeros(shape, dtype) for shape, dtype in zero_info)

    zeros_fn = jax.jit(shard_map(
        _mkz, mesh=mesh, in_specs=(), out_specs=(PartitionSpec("core"),) * n_outs,
        check_rep=False))

    _CACHED.update(nc=nc, fn=fn, zeros_fn=zeros_fn, in_names=in_names,
                   out_names=out_names)
    return _CACHED


def kernel(conv_local, conv_global, Wk, bk, Wq, bq, Wv, bv):
    C = _get_exec()
    bf = ml_dtypes.bfloat16

    xl = np.asarray(conv_local, dtype=np.float32)
    xg = np.asarray(conv_global, dtype=np.float32)
    wk = np.asarray(Wk, dtype=np.float32)
    wq = np.asarray(Wq, dtype=np.float32)
    wv = np.asarray(Wv, dtype=np.float32)
    bqv = np.asarray(bq, dtype=np.float32).reshape(D)
    bvv = np.asarray(bv, dtype=np.float32).reshape(1, D)

    # core c = 2*b + h: X^T for batch b (repeated per half), G^T for half h
    xt = np.ascontiguousarray(xl.astype(bf).transpose(0, 2, 1))      # [B, D, S]
    xt_cat = np.repeat(xt, 2, axis=0).reshape(NCORES * D, S)
    gt = np.ascontiguousarray(
        xg.astype(bf).reshape(NCORES, SQH, D).transpose(0, 2, 1))    # [8, D, SQH]
    gt_cat = gt.reshape(NCORES * D, SQH)

    # fused score weights in f32 on the host: M~[i,a] = (Wq Wk^T), b~ = Wk bq
    mt = (wq @ wk.T).astype(bf)                                      # [i, a]
    bt = (wk @ bqv).astype(np.float32).reshape(D, 1)
    mt_cat = np.tile(mt, (NCORES, 1))
    bt_cat = np.tile(bt, (NCORES, 1))
    wv_cat = np.tile(wv.astype(bf), (NCORES, 1))
    bv_cat = np.tile(bvv, (NCORES, 1))

    arrs = {"xt": xt_cat, "gt": gt_cat, "mt": mt_cat, "wv": wv_cat,
            "bt": bt_cat, "bv": bv_cat}
    inputs = [arrs[name] for name in C["in_names"]]
    zeros = C["zeros_fn"]()
    outs = C["fn"](*inputs, *zeros)
    out = np.asarray(outs[0]).astype(np.float32)
    return out.reshape(B, 2, SQH, D).reshape(B, S, D)


# revision 24
# speedup vs baseline: 1.1513x; 1.1513x over previous
"""Trainium2 Bass kernel for nn_AttentionMap (B=4, S=4096, D=256 full attention).

Sharding: 8 cores = 4 batches x 2 query-halves (data-parallel batch,
sequence-parallel over query rows, softmax rows stay whole per core).
No collectives: core c computes out[c//2, (c%2)*2048:(c%2+1)*2048, :]
from conv_local[c//2] and its conv_global slice.

Host-side preprocessing inside kernel() (same uploaded bytes, big device
savings):
  - X and G are uploaded TRANSPOSED (X^T [d, s], G^T [d, q]) so the device
    never runs the 96 PE transposes + 96 PSUM-drain copies the naive layout
    needs (they dominated the prologue).
  - The score weights are folded on the host in f32: M~ = Wq Wk^T [i, a]
    and b~ = Wk bq, so K and Q are never materialized; bk drops out
    entirely (softmax cancels a per-row constant).
  - All big tensors are cast to bf16 on the host (halves the axon-tunnel
    upload; the device would cast to bf16 for the PE anyway).

Device math (per core; every matmul contracts over the partition dim):
  Y^T[a, q] = M~T.T @ G^T + b~        (a-chunks of 128, q in tiles of 512)
  V[s, :256] = X^T-chunks.T @ Wv + bv ; V[s, 256:258] = 1  (ones-columns)
  per q-tile of 512 rows, software-pipelined with the previous tile's PV:
    S^T[s, q] = X^T-chunk.T @ Y^T-tile   (PSUM f32, 2 kv-chunks per 2-bank tile)
    p = exp(S^T / 16)  (ACT, bf16 out; no max-subtraction: scores ~ N(0,1)
        so bf16/f32 range is safe, softmax shift-invariance keeps results
        aligned with the reference)
    O_unnorm[q, 258] += p-chunk.T @ V-chunk  (4 PSUM accumulators; the
        ones-columns of V accumulate the softmax denominator)
    out = O_unnorm[:, :256] * reciprocal(O_unnorm[:, 256])  -> bf16 -> DMA
  The PV pass of q-tile i is interleaved between the S^T pairs of q-tile
  i+1 so the PE stays dense while ACT works through the exp backlog.

All matmuls are bf16 (fp8e4 DoubleRow was tried and is 1.9x faster on the
PE, but its ~3.6% operand quantization puts ~eps*w_max*|v| ~ 2-4e-2 absmax
error on rows where softmax concentrates - over the 2e-2 gate).

Output is bf16 (halves download), cast back to f32 on the host. The jitted
shard_map executable is built once and cached; donated output buffers are
created on-device instead of uploading host zeros.
"""

import os
import sys
from contextlib import ExitStack

import ml_dtypes
import numpy as np

for _p in ("/opt/trn_rl_repo", "/root/.axon_site/_ro/trn_rl_repo"):
    if _p not in sys.path and os.path.isdir(_p):
        sys.path.append(_p)

import concourse.mybir as mybir
import concourse.tile as tile
from concourse import bacc

B = 4
S = 4096          # kv sequence length (= full query length)
D = 256           # model dim = head dim
NCORES = 8
SQH = S // 2      # query rows per core (2048)
QT = 512          # query tile (moving free dim of the S^T matmuls)
NQT = SQH // QT   # 4
NSC = S // 128    # 32 kv chunks of 128
NDC = D // 128    # 2 d chunks of 128
VPAD = 2          # ones-columns appended to V (softmax denominator)
NXSEG = 2         # xt/gt DMA split for load/compute overlap (4KB packets)
F32 = mybir.dt.float32
BF16 = mybir.dt.bfloat16
I32 = mybir.dt.int32

# Schraudolph fast-exp on DVE for a subset of kv-chunks (ACT exp is the
# phase-2 wall otherwise): p ~= bitcast_f32(int32(s*EXPA + EXPB)) with
# max rel err ~3.1%; the softmax ratio cancels the common factor so the
# end-to-end absmax cost is a few 1e-3. Chunks with t % 4 == 2 go to DVE.
EXP_SCALE = 1.0 / 16.0          # 1/sqrt(D)
EXPA = float(2 ** 23) * float(np.log2(np.e)) * EXP_SCALE
EXPB = float(2 ** 23) * (127.0 - 0.0450)


def _dve_exp_chunk(t):
    return t % 4 == 2

_CACHED = {}


def build_program():
    nc = bacc.Bacc("TRN2", target_bir_lowering=False, debug=False)

    xt_d = nc.dram_tensor("xt", [D, S], BF16, kind="ExternalInput").ap()
    gt_d = nc.dram_tensor("gt", [D, SQH], BF16, kind="ExternalInput").ap()
    mt_d = nc.dram_tensor("mt", [D, D], BF16, kind="ExternalInput").ap()
    wv_d = nc.dram_tensor("wv", [D, D], BF16, kind="ExternalInput").ap()
    bt_d = nc.dram_tensor("bt", [D, 1], F32, kind="ExternalInput").ap()
    bv_d = nc.dram_tensor("bv", [1, D], F32, kind="ExternalInput").ap()
    out_d = nc.dram_tensor("out", [SQH, D], BF16, kind="ExternalOutput").ap()

    with tile.TileContext(nc) as tc, ExitStack() as ctx:
        Exp = mybir.ActivationFunctionType.Exp

        consts = ctx.enter_context(tc.tile_pool(name="consts", bufs=1))
        big = ctx.enter_context(tc.tile_pool(name="big", bufs=1))

        # ---- phase 2 SBUF residents ----
        xt = big.tile([128, NDC, S], BF16)          # X^T [d, s]
        gt = big.tile([128, NDC, SQH], BF16)        # G^T [d, q]
        yt = big.tile([128, NDC, SQH], BF16)        # M~T.T G^T + b~  [a, q]
        vt = big.tile([128, NSC, D + VPAD], BF16)   # V||1 [s, d+pad]

        mt_sb = consts.tile([128, NDC, D], BF16)    # M~T [i, a]
        wv_sb = consts.tile([128, NDC, D], BF16)
        bt_sb = consts.tile([128, NDC, 1], F32)     # b~ [a]
        ones1 = consts.tile([1, 128], BF16)
        ones1_f32 = consts.tile([1, 128], F32)
        vone_f32 = consts.tile([128, NSC, VPAD], F32)
        bv_bc = consts.tile([128, D], F32)

        # DMA across 3 queues (sync/scalar/gpsimd — the only DMA-capable
        # engines): gt first (it gates Y^T and thus the first score matmul),
        # weights on gpsimd, then xt in progressive 512-col segments so the
        # V projection can chase the stream
        bv_ld = consts.tile([1, D], F32, tag="bvl")
        nc.gpsimd.dma_start(bv_ld[:], bv_d[:])
        for kc in range(NDC):
            nc.gpsimd.dma_start(mt_sb[:, kc, :], mt_d[kc * 128:(kc + 1) * 128, :])
            nc.gpsimd.dma_start(wv_sb[:, kc, :], wv_d[kc * 128:(kc + 1) * 128, :])
            nc.gpsimd.dma_start(bt_sb[:, kc, :], bt_d[kc * 128:(kc + 1) * 128, :])
        xq = (nc.sync, nc.scalar)
        for sg in range(4):
            for kc in range(NDC):
                xq[kc].dma_start(
                    gt[:, kc, sg * 512:(sg + 1) * 512],
                    gt_d[kc * 128:(kc + 1) * 128, sg * 512:(sg + 1) * 512])
        for sg in range(8):
            for kc in range(NDC):
                xq[kc].dma_start(
                    xt[:, kc, sg * 512:(sg + 1) * 512],
                    xt_d[kc * 128:(kc + 1) * 128, sg * 512:(sg + 1) * 512])
        bv_rhs = consts.tile([1, D], BF16, tag="bvc")
        nc.vector.tensor_copy(bv_rhs[:], bv_ld[:])
        nc.vector.memset(ones1_f32[:], 1.0)
        nc.vector.tensor_copy(ones1[:], ones1_f32[:])
        nc.vector.memset(vone_f32[:], 1.0)

        p01 = ExitStack()
        with p01:
            mmp = p01.enter_context(tc.tile_pool(name="mmp", bufs=4, space="PSUM"))

            # bv broadcast across partitions via a K=1 matmul
            psb = mmp.tile([128, D], F32, tag="proj")
            nc.tensor.matmul(psb[:], ones1[:], bv_rhs[:], start=True, stop=True)
            nc.vector.tensor_copy(bv_bc[:], psb[:])
            nc.vector.tensor_copy(vt[:, :, D:D + VPAD], vone_f32[:])

            def emit_yt_block(nt):
                # Y^T[a, q] = sum_i M~T[i, a-block] @ G^T[i, q] + b~[a]
                for dc in range(NDC):
                    psy = mmp.tile([128, 512], F32, tag="proj", name="psy")
                    for ic in range(NDC):
                        nc.tensor.matmul(
                            psy[:],
                            mt_sb[:, ic, dc * 128:(dc + 1) * 128],
                            gt[:, ic, nt * 512:(nt + 1) * 512],
                            start=(ic == 0), stop=(ic == NDC - 1),
                        )
                    nc.vector.tensor_scalar_add(
                        yt[:, dc, nt * 512:(nt + 1) * 512], psy[:], bt_sb[:, dc, :])

            # Interleave: yt block 0 first (gates q-tile 0 scores), V chunks
            # chase the xt DMA stream, remaining yt blocks spread between
            emit_yt_block(0)
            for t in range(NSC):
                psv = mmp.tile([128, D], F32, tag="proj", name="psv")
                for kc in range(NDC):
                    nc.tensor.matmul(
                        psv[:],
                        xt[:, kc, t * 128:(t + 1) * 128],
                        wv_sb[:, kc, :],
                        start=(kc == 0), stop=(kc == NDC - 1),
                    )
                nc.vector.tensor_add(vt[:, t, 0:D], psv[:], bv_bc[:])
                if t in (7, 15, 23):
                    emit_yt_block(t // 8 + 1)

        # ---- phase 2: attention, PV software-pipelined one q-tile behind ----
        esp = ctx.enter_context(tc.tile_pool(name="esp", bufs=2))
        stp = ctx.enter_context(tc.tile_pool(name="stp", bufs=3, space="PSUM"))
        pvp = ctx.enter_context(tc.tile_pool(name="pvp", bufs=1, space="PSUM"))
        osb_p = ctx.enter_context(tc.tile_pool(name="osb", bufs=4))

        inv_sqrt_d = 1.0 / float(np.sqrt(D))
        nqs = QT // 128

        yip = ctx.enter_context(tc.tile_pool(name="yip", bufs=3))

        def emit_scores_pair(es, q0, tp):
            # single-bank psum tiles (bufs=4): 4-deep PE->ACT pipelining; a
            # 2-bank/bufs=2 pair tile ping-pongs PE and ACT (~12 us/qtile)
            for sub in range(2):
                t = 2 * tp + sub
                ps = stp.tile([128, QT], F32, tag="st")
                for kc in range(NDC):
                    nc.tensor.matmul(
                        ps[:],
                        xt[:, kc, t * 128:(t + 1) * 128],
                        yt[:, kc, q0:q0 + QT],
                        start=(kc == 0), stop=(kc == NDC - 1),
                    )
                if _dve_exp_chunk(t):
                    yi = yip.tile([128, QT], I32, tag="yi")
                    nc.vector.tensor_scalar(
                        yi[:], ps[:], EXPA, EXPB,
                        op0=mybir.AluOpType.mult, op1=mybir.AluOpType.add)
                    nc.vector.tensor_copy(es[:, t, :], yi[:].bitcast(F32))
                else:
                    nc.scalar.activation(es[:, t, :], ps[:], Exp,
                                         scale=inv_sqrt_d)

        def emit_pv_pair(accs, es, tp):
            for qs in range(nqs):
                for t in (2 * tp, 2 * tp + 1):
                    nc.tensor.matmul(
                        accs[qs][:],
                        es[:, t, qs * 128:(qs + 1) * 128],
                        vt[:, t, :],
                        start=(t == 0), stop=(t == NSC - 1),
                    )

        def emit_finalize(accs, q0):
            for qs in range(nqs):
                acc = accs[qs]
                osb = osb_p.tile([128, D], BF16, tag="osb")
                rec = osb_p.tile([128, 1], F32, tag="rec")
                nc.vector.reciprocal(rec[:], acc[:, D:D + 1])
                nc.vector.tensor_scalar_mul(osb[:], acc[:, 0:D], rec[:])
                nc.gpsimd.dma_start(
                    out_d[q0 + qs * 128:q0 + (qs + 1) * 128, :], osb[:])

        # flat slot pipeline: PV trails scores by PVLAG slots (same q-tile),
        # so the end-of-kernel drain is PVLAG slots instead of a whole tile
        PVLAG = 2
        NSLOT = NSC // 2
        state = {}  # qi -> (es, accs, q0)
        for s in range(NQT * NSLOT + PVLAG):
            if s < NQT * NSLOT:
                qi, tp = divmod(s, NSLOT)
                if tp == 0:
                    es = esp.tile([128, NSC, QT], BF16, tag="es", name="es")
                    accs = [pvp.tile([128, D + VPAD], F32, tag=f"acc{qs}",
                                     name=f"acc{qs}") for qs in range(nqs)]
                    state[qi] = (es, accs, qi * QT)
                emit_scores_pair(state[qi][0], state[qi][2], tp)
            if s >= PVLAG:
                qj, tq = divmod(s - PVLAG, NSLOT)
                emit_pv_pair(state[qj][1], state[qj][0], tq)
                if tq == NSLOT - 1:
                    emit_finalize(state[qj][1], state[qj][2])
                    del state[qj]

    nc.compile()
    return nc


def _get_exec():
    if "fn" in _CACHED:
        return _CACHED

    import jax
    import jax.numpy as jnp
    from jax.sharding import Mesh, PartitionSpec
    from jax.experimental.shard_map import shard_map
    from concourse import bass2jax

    nc = build_program()
    bass2jax.install_neuronx_cc_hook()

    partition_name = nc.partition_id_tensor.name if nc.partition_id_tensor else None
    in_names, out_names, out_avals = [], [], []
    for alloc in nc.m.functions[0].allocations:
        if not isinstance(alloc, mybir.MemoryLocationSet):
            continue
        name = alloc.memorylocations[0].name
        if alloc.kind == "ExternalInput":
            if name != partition_name:
                in_names.append(name)
        elif alloc.kind == "ExternalOutput":
            out_names.append(name)
            out_avals.append(jax.core.ShapedArray(
                tuple(alloc.tensor_shape), mybir.dt.np(alloc.dtype)))
    n_params = len(in_names)
    n_outs = len(out_avals)
    in_names_all = in_names + out_names
    if partition_name is not None:
        in_names_all.append(partition_name)
    donate = tuple(range(n_params, n_params + n_outs))

    def _body(*args):
        operands = list(args)
        if partition_name is not None:
            operands.append(bass2jax.partition_id_tensor())
        return tuple(bass2jax._bass_exec_p.bind(
            *operands,
            out_avals=tuple(out_avals),
            in_names=tuple(in_names_all),
            out_names=tuple(out_names),
            lowering_input_output_aliases=(),
            sim_require_finite=True,
            sim_require_nnan=True,
            nc=nc,
        ))

    devices = jax.devices()[:NCORES]
    mesh = Mesh(np.asarray(devices), ("core",))
    in_specs = (PartitionSpec("core"),) * (n_params + n_outs)
    out_specs = (PartitionSpec("core"),) * n_outs
    fn = jax.jit(
        shard_map(_body, mesh=mesh, in_specs=in_specs, out_specs=out_specs,
                  check_rep=False),
        donate_argnums=donate, keep_unused=True,
    )

    zero_info = [(tuple(a.shape), a.dtype) for a in out_avals]

    def _mkz():
        return tuple(jnp.zeros(shape, dtype) for shape, dtype in zero_info)

    zeros_fn = jax.jit(shard_map(
        _mkz, mesh=mesh, in_specs=(), out_specs=(PartitionSpec("core"),) * n_outs,
        check_rep=False))

    _CACHED.update(nc=nc, fn=fn, zeros_fn=zeros_fn, in_names=in_names,
                   out_names=out_names)
    return _CACHED


def kernel(conv_local, conv_global, Wk, bk, Wq, bq, Wv, bv):
    C = _get_exec()
    bf = ml_dtypes.bfloat16

    xl = np.asarray(conv_local, dtype=np.float32)
    xg = np.asarray(conv_global, dtype=np.float32)
    wk = np.asarray(Wk, dtype=np.float32)
    wq = np.asarray(Wq, dtype=np.float32)
    wv = np.asarray(Wv, dtype=np.float32)
    bqv = np.asarray(bq, dtype=np.float32).reshape(D)
    bvv = np.asarray(bv, dtype=np.float32).reshape(1, D)

    # core c = 2*b + h: X^T for batch b (repeated per half), G^T for half h
    xt = np.ascontiguousarray(xl.astype(bf).transpose(0, 2, 1))      # [B, D, S]
    xt_cat = np.repeat(xt, 2, axis=0).reshape(NCORES * D, S)
    gt = np.ascontiguousarray(
        xg.astype(bf).reshape(NCORES, SQH, D).transpose(0, 2, 1))    # [8, D, SQH]
    gt_cat = gt.reshape(NCORES * D, SQH)

    # fused score weights in f32 on the host: M~[i,a] = (Wq Wk^T), b~ = Wk bq
    mt = (wq @ wk.T).astype(bf)                                      # [i, a]
    bt = (wk @ bqv).astype(np.float32).reshape(D, 1)
    mt_cat = np.tile(mt, (NCORES, 1))
    bt_cat = np.tile(bt, (NCORES, 1))
    wv_cat = np.tile(wv.astype(bf), (NCORES, 1))
    bv_cat = np.tile(bvv, (NCORES, 1))

    arrs = {"xt": xt_cat, "gt": gt_cat, "mt": mt_cat, "wv": wv_cat,
            "bt": bt_cat, "bv": bv_cat}
    inputs = [arrs[name] for name in C["in_names"]]
    zeros = C["zeros_fn"]()
    outs = C["fn"](*inputs, *zeros)
    out = np.asarray(outs[0]).astype(np.float32)
    return out.reshape(B, 2, SQH, D).reshape(B, S, D)
